# revision 4
# baseline (speedup 1.0000x reference)
"""AutoLevel (non-differentiable) Trainium2 Bass kernel, v2.

Computes, per image b of a [B, 3, H, W] f32 batch:
    y       = rgb2yuv[0] . image[b]            (luma)
    blkpt   = percentile(y, 1.0)
    whtpt   = percentile(y, 99.0)
    mult    = min(1 / (whtpt - blkpt), 1.5)
    out[b]  = clip((image[b] - blkpt) * mult, 0, 1)

Sharding: data-parallel over batch. 16 images / 8 cores = 2 images/core.

v2 strategy (vs the exact-rank v1): the correctness gate is rel_err < 2e-2,
so percentiles only need ~1e-3 absolute accuracy. That allows:

  1. Single HBM read: each f32 chunk is cast to a bf16 SBUF copy (ScalarE)
     as it arrives; the output transform re-reads the bf16 copy instead of
     DRAM. HBM traffic drops 75.5 MB -> 50.4 MB per core (the roofline).
  2. Half-resolution luma: y' = B*(wb/wg) + G + R*(wr/wg) on even columns
     only (524288 samples/image) — half the VectorE madd cost. Percentiles
     of y are wg * percentiles of y'; half-sampling adds ~5e-4 noise.
  3. Percentile via counts, not ranks:
       a. two-level 8-threshold grid on a 1/8-strided bf16 sample (65536
          values): two matmul round-trips localize v0 to +-0.011 + sample
          noise (<< +-0.04).
       b. three exact VectorE count passes over y' at v0-D, v0, v0+D
          (D=0.04), cross-partition reduce via ones-matmul, then
          inverse-quadratic interpolation through the three exact CDF
          points. Residual error ~2e-4 in y units.
  4. Transform: ScalarE Relu(mult*x - mult*blkpt) from the bf16 copy, then
     VectorE min(.,1), chunk-streamed to DRAM.

Engine budget/core: DVE ~83us, ACT ~89us, DMA ~144us -> DMA-bound.

A dbg output carries a bracket guard (c(v0-D) <= k <= c(v0+D)) and the
interpolated percentiles; if the guard fails or values are non-finite the
host recomputes that image's percentiles in numpy (never fires for data in
the expected distribution family; pure safety net).
"""

import sys

if "/opt/trn_rl_repo" not in sys.path:
    sys.path.insert(0, "/opt/trn_rl_repo")

import numpy as np

P = 128
F = 8192                # free elems of one 1024x1024 plane on 128 partitions
TC = 2048               # load/transform chunk width
HTC = TC // 2           # half-res luma chunk width
HF = F // 2             # half-res luma plane width
N = 1024 * 1024         # pixels per image
NH = N // 2             # half-res luma samples
NS = NH // 8            # strided sample values (65536)
BLKP, WHTP = 1.0, 99.0
MAX_MULT = 1.5
LO0 = -0.01             # level-1 grid origin (y' units)
STEP0 = 0.2             # level-1 grid step; 8 thresholds at LO0+(j+1)*STEP0
STEP1 = STEP0 / 9.0     # level-2 grid step
DW = 0.04               # exact-count half window (y' units)
IMGS_PER_CORE = 2
NCORES = 8

_CACHE = {}


def _idx(p):
    return p / 100.0 * (N - 1)


def _build(w_r, w_g, w_b, repeat=1):
    import concourse.bass as bass
    import concourse.bacc as bacc
    import concourse.mybir as mybir
    import concourse.tile as tile

    f32 = mybir.dt.float32
    bf16 = mybir.dt.bfloat16
    Op = mybir.AluOpType
    Act = mybir.ActivationFunctionType

    c_bg = float(np.float32(w_b / w_g))
    c_rg = float(np.float32(w_r / w_g))
    S = float(np.float32(w_g))

    # count targets: full-image fractional index, scaled to half-res (exact
    # counts) and 1/16 (strided sample counts)
    k_h = {0: _idx(BLKP) / 2.0, 1: _idx(WHTP) / 2.0}
    k_s = {0: _idx(BLKP) / 16.0, 1: _idx(WHTP) / 16.0}

    nc = bacc.Bacc("TRN2", target_bir_lowering=False, debug=False,
                   enable_asserts=False, num_devices=NCORES)

    img = nc.dram_tensor("img", [IMGS_PER_CORE, 3, P, F], f32,
                         kind="ExternalInput").ap()
    outt = nc.dram_tensor("out", [IMGS_PER_CORE, 3, P, F], f32,
                          kind="ExternalOutput").ap()
    dbg = nc.dram_tensor("dbg", [IMGS_PER_CORE, 8], f32,
                         kind="ExternalOutput").ap()

    with tile.TileContext(nc) as tc:
        with (
            tc.tile_pool(name="chunks", bufs=4) as chk,
            tc.tile_pool(name="cu", bufs=2) as cup,
            tc.tile_pool(name="big", bufs=1) as big,
            tc.tile_pool(name="sm", bufs=1) as sm,
            tc.tile_pool(name="ps_a", bufs=1, space="PSUM") as ppa,
            tc.tile_pool(name="ps_b", bufs=1, space="PSUM") as ppb,
        ):
            ones = sm.tile([P, P], f32, tag="ones")
            nc.vector.memset(ones[:], 1.0)
            # (j+1)*STEP1 row vector for level-2 grid construction
            i8s = sm.tile([P, 8], f32, tag="i8s")
            for j in range(8):
                nc.vector.memset(i8s[:, j:j + 1], (j + 1) * STEP1)
            # per-channel exact-count targets [k_blk, k_wht]
            k2c = sm.tile([P, 2], f32, tag="k2c")
            for ch in (0, 1):
                nc.vector.memset(k2c[:, ch:ch + 1], k_h[ch])

            for rep in range(repeat):
                xb, yh, scr, ys, st, ps = {}, {}, {}, {}, {}, {}
                for i in range(IMGS_PER_CORE):
                    xb[i] = [big.tile([P, F], bf16, tag=f"xb{i}{c}",
                                       name=f"xb{i}{c}")
                             for c in range(3)]
                    yh[i] = big.tile([P, HF], f32, tag=f"yh{i}", name=f"yh{i}")
                    scr[i] = big.tile([P, HF], bf16, tag=f"scr{i}", name=f"scr{i}")
                    ys[i] = sm.tile([P, 512], bf16, tag=f"ys{i}", name=f"ys{i}")
                    st[i] = sm.tile([P, 96], f32, tag=f"st{i}", name=f"st{i}")
                    ps[i] = ppa if i == 0 else ppb

                # st columns (all [P,1] f32):
                # 0:8   level-1 per-partition counts
                # 8:24  level-2 per-partition counts (0:8 blk, 8:16 wht)
                # 24,25 s (level-1 sel) blk,wht ; 26,27 vlo blk,wht
                # 28,29 s2 blk,wht
                # 30,31 v0 blk,wht  (= m thresholds m2)
                # 32:34 a2 ; 34:36 b2
                # 36:42 cnt6 accum [caB caW cmB cmW cbB cbW]
                # 42:48 cc (reduced counts, same order)
                # 48:50 dman ; 50:52 dbmn ; 52:54 dban
                # 54:56 r1 ; 56:58 r2 ; 58:60 r3
                # 60:62 d1 ; 62:64 sl2 ; 64:66 d2
                # 66:68 e1 ; 68:70 e2 ; 70:72 q
                # 72:74 v* (interp percentile, y' units)
                # 74:76 guard sum ; 76,77 blkpt,mfac ; 78 beta ; 79 dv
                def phase_a(i):
                    for h in range(4):
                        cols = slice(h * TC, (h + 1) * TC)
                        hcols = slice(h * HTC, (h + 1) * HTC)
                        bB = chk.tile([P, TC], f32, tag="c", name="bB")
                        nc.sync.dma_start(out=bB[:], in_=img[i, 2, :, cols])
                        bG = chk.tile([P, TC], f32, tag="c", name="bG")
                        nc.sync.dma_start(out=bG[:], in_=img[i, 1, :, cols])
                        bR = chk.tile([P, TC], f32, tag="c", name="bR")
                        nc.sync.dma_start(out=bR[:], in_=img[i, 0, :, cols])
                        nc.scalar.activation(out=xb[i][2][:, cols], in_=bB[:],
                                             func=Act.Copy, scale=1.0, bias=0.0)
                        nc.vector.scalar_tensor_tensor(
                            out=yh[i][:, hcols], in0=bB[:, ::2], scalar=c_bg,
                            in1=bG[:, ::2], op0=Op.mult, op1=Op.add)
                        nc.scalar.activation(out=xb[i][1][:, cols], in_=bG[:],
                                             func=Act.Copy, scale=1.0, bias=0.0)
                        nc.vector.scalar_tensor_tensor(
                            out=yh[i][:, hcols], in0=bR[:, ::2], scalar=c_rg,
                            in1=yh[i][:, hcols], op0=Op.mult, op1=Op.add)
                        nc.scalar.activation(out=xb[i][0][:, cols], in_=bR[:],
                                             func=Act.Copy, scale=1.0, bias=0.0)

                def phase_s(i):
                    s = st[i]
                    nc.vector.tensor_copy(out=ys[i][:], in_=yh[i][:, ::8])
                    sscr = scr[i][:, 0:512]
                    for j in range(8):
                        nc.vector.tensor_scalar(
                            out=sscr, in0=ys[i][:],
                            scalar1=float(LO0 + (j + 1) * STEP0), scalar2=None,
                            op0=Op.is_lt, op1=Op.add,
                            accum_out=s[:, j:j + 1])
                    p8 = ps[i].tile([P, 8], f32, tag="l1")
                    nc.tensor.matmul(p8[:], ones[:], s[:, 0:8],
                                     start=True, stop=True)
                    s8 = sm.tile([P, 8], f32, tag=f"s8_{i}")
                    for ch in (0, 1):
                        nc.vector.tensor_scalar(
                            out=s8[:], in0=p8[:], scalar1=k_s[ch],
                            scalar2=None, op0=Op.is_le, op1=Op.add,
                            accum_out=s[:, 24 + ch:25 + ch])
                        nc.vector.tensor_scalar(
                            out=s[:, 26 + ch:27 + ch],
                            in0=s[:, 24 + ch:25 + ch], scalar1=STEP0,
                            scalar2=LO0, op0=Op.mult, op1=Op.add)
                    t8 = {}
                    for ch in (0, 1):
                        t8[ch] = sm.tile([P, 8], f32, tag=f"t8_{i}{ch}",
                                         name=f"t8_{i}{ch}")
                        nc.vector.tensor_scalar(
                            out=t8[ch][:], in0=i8s[:],
                            scalar1=s[:, 26 + ch:27 + ch], scalar2=None,
                            op0=Op.add)
                    for ch in (0, 1):
                        for j in range(8):
                            nc.vector.tensor_scalar(
                                out=sscr, in0=ys[i][:],
                                scalar1=t8[ch][:, j:j + 1], scalar2=None,
                                op0=Op.is_lt, op1=Op.add,
                                accum_out=s[:, 8 + 8 * ch + j:9 + 8 * ch + j])
                    p16 = ps[i].tile([P, 16], f32, tag="l2")
                    nc.tensor.matmul(p16[:], ones[:], s[:, 8:24],
                                     start=True, stop=True)
                    for ch in (0, 1):
                        nc.vector.tensor_scalar(
                            out=s8[:], in0=p16[:, 8 * ch:8 * ch + 8],
                            scalar1=k_s[ch], scalar2=None,
                            op0=Op.is_le, op1=Op.add,
                            accum_out=s[:, 28 + ch:29 + ch])
                        # v0 = vlo + (s2 + 0.5)*STEP1
                        nc.vector.tensor_scalar(
                            out=s[:, 30 + ch:31 + ch],
                            in0=s[:, 28 + ch:29 + ch], scalar1=STEP1,
                            scalar2=0.5 * STEP1, op0=Op.mult, op1=Op.add)
                    nc.vector.tensor_add(out=s[:, 30:32], in0=s[:, 30:32],
                                         in1=s[:, 26:28])
                    nc.vector.tensor_scalar(out=s[:, 32:34], in0=s[:, 30:32],
                                            scalar1=DW, scalar2=None,
                                            op0=Op.subtract)
                    nc.vector.tensor_scalar(out=s[:, 34:36], in0=s[:, 30:32],
                                            scalar1=DW, scalar2=None,
                                            op0=Op.add)

                def phase_c(i):
                    s = st[i]
                    thr = {0: s[:, 32:34], 1: s[:, 30:32], 2: s[:, 34:36]}
                    for t in range(3):          # a, m, b
                        for ch in (0, 1):
                            nc.vector.tensor_scalar(
                                out=scr[i][:], in0=yh[i][:],
                                scalar1=thr[t][:, ch:ch + 1], scalar2=None,
                                op0=Op.is_lt, op1=Op.add,
                                accum_out=s[:, 36 + 2 * t + ch:37 + 2 * t + ch])
                    p6 = ps[i].tile([P, 6], f32, tag="c6")
                    nc.tensor.matmul(p6[:], ones[:], s[:, 36:42],
                                     start=True, stop=True)
                    cc = s[:, 42:48]
                    nc.vector.tensor_copy(out=cc, in_=p6[:])
                    ca2, cm2, cb2 = cc[:, 0:2], cc[:, 2:4], cc[:, 4:6]
                    a2, m2, b2 = s[:, 32:34], s[:, 30:32], s[:, 34:36]
                    nc.vector.tensor_sub(out=s[:, 48:50], in0=cm2, in1=ca2)
                    nc.vector.tensor_sub(out=s[:, 50:52], in0=cb2, in1=cm2)
                    nc.vector.tensor_sub(out=s[:, 52:54], in0=cb2, in1=ca2)
                    nc.vector.reciprocal(out=s[:, 54:56], in_=s[:, 48:50])
                    nc.vector.reciprocal(out=s[:, 56:58], in_=s[:, 50:52])
                    nc.vector.reciprocal(out=s[:, 58:60], in_=s[:, 52:54])
                    nc.vector.tensor_scalar(out=s[:, 60:62], in0=s[:, 54:56],
                                            scalar1=DW, scalar2=None,
                                            op0=Op.mult)
                    nc.vector.tensor_scalar(out=s[:, 62:64], in0=s[:, 56:58],
                                            scalar1=DW, scalar2=None,
                                            op0=Op.mult)
                    nc.vector.tensor_sub(out=s[:, 64:66], in0=s[:, 62:64],
                                         in1=s[:, 60:62])
                    nc.vector.tensor_mul(out=s[:, 64:66], in0=s[:, 64:66],
                                         in1=s[:, 58:60])
                    nc.vector.tensor_sub(out=s[:, 66:68], in0=k2c[:], in1=ca2)
                    nc.vector.tensor_sub(out=s[:, 68:70], in0=k2c[:], in1=cm2)
                    # v* = a + e1*d1 + e1*e2*d2
                    nc.vector.tensor_mul(out=s[:, 70:72], in0=s[:, 66:68],
                                         in1=s[:, 60:62])
                    nc.vector.tensor_add(out=s[:, 72:74], in0=a2,
                                         in1=s[:, 70:72])
                    nc.vector.tensor_mul(out=s[:, 70:72], in0=s[:, 66:68],
                                         in1=s[:, 68:70])
                    nc.vector.tensor_mul(out=s[:, 70:72], in0=s[:, 70:72],
                                         in1=s[:, 64:66])
                    nc.vector.tensor_add(out=s[:, 72:74], in0=s[:, 72:74],
                                         in1=s[:, 70:72])
                    # bracket guard: (ca <= k) + (k <= cb) == 2
                    nc.vector.tensor_tensor(out=s[:, 74:76], in0=ca2,
                                            in1=k2c[:], op=Op.is_le)
                    nc.vector.tensor_tensor(out=s[:, 70:72], in0=k2c[:],
                                            in1=cb2, op=Op.is_le)
                    nc.vector.tensor_add(out=s[:, 74:76], in0=s[:, 74:76],
                                         in1=s[:, 70:72])
                    # blkpt = S*vB ; mfac = min(1/(S*(vW-vB)), MAX_MULT)
                    nc.vector.tensor_sub(out=s[:, 79:80], in0=s[:, 73:74],
                                         in1=s[:, 72:73])
                    nc.vector.reciprocal(out=s[:, 79:80], in_=s[:, 79:80])
                    nc.vector.tensor_scalar(out=s[:, 77:78], in0=s[:, 79:80],
                                            scalar1=1.0 / S, scalar2=MAX_MULT,
                                            op0=Op.mult, op1=Op.min)
                    nc.vector.tensor_scalar(out=s[:, 76:77], in0=s[:, 72:73],
                                            scalar1=S, scalar2=None,
                                            op0=Op.mult)
                    nc.vector.scalar_tensor_tensor(
                        out=s[:, 78:79], in0=s[:, 76:77], scalar=-1.0,
                        op0=Op.mult, op1=Op.mult, in1=s[:, 77:78])
                    nc.sync.dma_start(out=dbg[i, 0:2], in_=s[0:1, 74:76])
                    nc.sync.dma_start(out=dbg[i, 2:4], in_=s[0:1, 72:74])
                    nc.sync.dma_start(out=dbg[i, 4:6], in_=s[0:1, 42:44])
                    nc.sync.dma_start(out=dbg[i, 6:8], in_=s[0:1, 46:48])

                def phase_t(i):
                    s = st[i]
                    mfac = s[:, 77:78]
                    beta = s[:, 78:79]
                    for c in range(3):
                        for h in range(4):
                            cols = slice(h * TC, (h + 1) * TC)
                            cu = cup.tile([P, TC], f32, tag="cu", name="cu")
                            nc.scalar.activation(
                                out=cu[:], in_=xb[i][c][:, cols],
                                func=Act.Relu, bias=beta, scale=mfac)
                            nc.vector.tensor_scalar(
                                out=cu[:], in0=cu[:], scalar1=1.0,
                                scalar2=None, op0=Op.min)
                            nc.sync.dma_start(out=outt[i, c, :, cols],
                                              in_=cu[:])

                phase_a(0)
                phase_s(0)
                phase_c(0)
                phase_a(1)
                phase_t(0)
                phase_s(1)
                phase_c(1)
                phase_t(1)

    nc.compile()
    return nc


def _get_nc(w_r, w_g, w_b):
    key = (round(float(w_r), 9), round(float(w_g), 9), round(float(w_b), 9))
    if key not in _CACHE:
        _CACHE[key] = _build(w_r, w_g, w_b)
    return _CACHE[key]


def _host_fallback(img_b):
    """Exact numpy recompute for one image [3, H, W]; safety net only."""
    w = np.array([0.299, 0.587, 0.114], dtype=np.float32)
    y = np.einsum("j,jhw->hw", w, img_b.astype(np.float32))
    yf = np.sort(y.reshape(-1))
    def pct(p):
        idx = p / 100.0 * (N - 1)
        i0 = int(np.floor(idx))
        fr = idx - i0
        return yf[i0] * (1 - fr) + yf[i0 + 1] * fr
    b, wht = pct(BLKP), pct(WHTP)
    m = min(1.0 / (wht - b), MAX_MULT)
    return np.clip((img_b - b) * m, 0.0, 1.0).astype(np.float32)


def kernel(image, rgb2yuv):
    from concourse.bass_utils import run_bass_kernel_spmd

    image = np.ascontiguousarray(np.asarray(image, dtype=np.float32))
    rgb2yuv = np.asarray(rgb2yuv, dtype=np.float32)
    B, C, H, W = image.shape
    assert (C, H, W) == (3, 1024, 1024) and B == NCORES * IMGS_PER_CORE

    w_r, w_g, w_b = (float(rgb2yuv[0, 0]), float(rgb2yuv[0, 1]),
                     float(rgb2yuv[0, 2]))
    nc = _get_nc(w_r, w_g, w_b)

    shards = image.reshape(NCORES, IMGS_PER_CORE, 3, P, F)
    in_maps = [{"img": shards[c]} for c in range(NCORES)]
    res = run_bass_kernel_spmd(nc, in_maps, list(range(NCORES))).results

    out = np.empty((B, 3, H, W), dtype=np.float32)
    for c in range(NCORES):
        o = res[c]["out"].reshape(IMGS_PER_CORE, 3, H, W)
        d = res[c]["dbg"]
        for i in range(IMGS_PER_CORE):
            b = c * IMGS_PER_CORE + i
            guard = d[i, 0:2]
            vstar = d[i, 2:4]
            ok = (np.all(np.abs(guard - 2.0) < 0.5)
                  and np.all(np.isfinite(vstar)))
            out[b] = o[i] if ok else _host_fallback(image[b])
    return out
